# revision 8
# baseline (speedup 1.0000x reference)
"""BagOfFeaturesModel Trainium2 kernel (8 NeuronCores, SPMD).

Strategy (tensor-parallel vocab-range sharding, per the spec hint's second
variant): core c owns vocab rows [c*12500, (c+1)*12500).  The host routes each
token to the core owning its feature (pure data movement), and within each
core orders tokens by (occurrence-rank within segment, segment), padding each
rank group to a whole 1024-token window.  That makes every 1024-token scatter
window duplicate-free, so the Q7 dma_scatter_add ucode (whose CCE
read-modify-write races on duplicate indices *within* a call) is exact; K
rotating accumulator copies keep cross-window adds race-free while still
overlapping.

Launch 1 (per core): dma_gather (int16 idx) from the 12500-row vocab shard ->
fp8 dropout-mask multiply (mask = jax threefry key(42), input-independent) ->
dma_scatter_add of z rows into K sums copies + 1.0 into K counts copies.

Launch 2 (per core): reduce the 8*K partial copies for the core's 2048-segment
range, mean = sums/max(counts,1), out^T = W @ mean^T + b on the PE, returning
[128, 2048]; the host reassembles [16384, 128].
"""
import functools
import sys

import numpy as np

if "/opt/trn_rl_repo" not in sys.path:
    sys.path.insert(0, "/opt/trn_rl_repo")


def make_nc():
    import concourse.bacc as bacc

    return bacc.Bacc()


T = 2_000_000
V = 100_000
E = 128
S = 16_384
O = 128
NCORES = 8
VS = V // NCORES  # 12_500 vocab rows per core
WIN = 2048  # tokens per gather/scatter window (single_packet=False path)
K = 4  # rotating accumulator copies
SEG_C = S // NCORES  # 2048 segments per core in launch 2
S_EXT = S + WIN  # scatter space incl. per-window dump rows for padding
ROW = 192  # scatter row: 128 emb + 1 count + 63 pad (768 B, /256 aligned)

_prog_cache = {}


def _ml_fp8():
    import ml_dtypes

    return ml_dtypes.float8_e4m3


@functools.lru_cache(maxsize=1)
def _mask_fp8():
    """Reference dropout mask (threefry key 42, keep-prob 0.5) scaled by 2,
    as float8_e4m3 {0.0, 2.0}. Input-independent constant of the problem."""
    import jax

    with jax.default_device(jax.devices("cpu")[0]):
        keep = jax.random.bernoulli(jax.random.key(42), 0.5, (T, E))
        keep = np.asarray(keep)
    return (keep.astype(np.float32) * 2.0).astype(_ml_fp8())


def _to_table(flat16: np.ndarray) -> np.ndarray:
    """Token flat-i -> idx table slot [i % 16, i // 16], replicated 8x to 128
    partitions (one copy per Q7 core)."""
    n = flat16.shape[0]
    t16 = flat16.reshape(n // 16, 16).T.copy()
    return np.tile(t16, (8, 1))


def _route_tokens(features: np.ndarray, indices: np.ndarray):
    """Assign tokens to cores by vocab range; order each core's tokens by
    (rank-within-segment, segment); pad rank groups to WIN multiples.

    Returns (nw, per-core list of (tok_positions, gidx16, sidx16)) where
    padded slots have tok_position == -1, gidx 0, sidx -1."""
    f = features.astype(np.int64)
    g = indices.astype(np.int64)
    core = f // VS
    order0 = np.lexsort((g, core))  # stable: by core, then segment
    cs = core[order0]
    ss = g[order0]
    newrun = np.r_[True, (cs[1:] != cs[:-1]) | (ss[1:] != ss[:-1])]
    idx = np.arange(T)
    starts = np.maximum.accumulate(np.where(newrun, idx, 0))
    rank = idx - starts
    # final order: core, then rank, then segment (stable keeps seg order)
    fin = np.lexsort((rank, cs))
    order = order0[fin]
    cs2 = cs[fin]
    rk2 = rank[fin]

    per_core = []
    lens = []
    for c in range(NCORES):
        sel = cs2 == c
        toks = order[sel]
        rks = rk2[sel]
        # split into rank groups, pad each to WIN multiple
        pieces = []
        if toks.size:
            bounds = np.r_[0, np.nonzero(rks[1:] != rks[:-1])[0] + 1, toks.size]
            for a, b in zip(bounds[:-1], bounds[1:]):
                grp = toks[a:b]
                pad = (-grp.size) % WIN
                if pad:
                    grp = np.r_[grp, np.full(pad, -1, np.int64)]
                pieces.append(grp)
        toks_p = np.concatenate(pieces) if pieces else np.zeros(0, np.int64)
        per_core.append(toks_p)
        lens.append(toks_p.size)
    nw = (max(lens) + WIN - 1) // WIN
    out = []
    for c in range(NCORES):
        toks_p = per_core[c]
        full = np.full(nw * WIN, -1, np.int64)
        full[: toks_p.size] = toks_p
        real = full >= 0
        gi = np.zeros(nw * WIN, np.int16)
        gi[real] = (f[full[real]] - c * VS).astype(np.int16)
        si = (S + (np.arange(nw * WIN) % WIN)).astype(np.int16)
        si[real] = g[full[real]].astype(np.int16)
        out.append((full, gi, si))
    return nw, out


def _build_launch1(nw: int):
    import concourse.mybir as mybir
    from concourse import tile

    nc = make_nc()
    emb_d = nc.declare_dram_parameter("embs", [VS, E], mybir.dt.float32, isOutput=False)
    gt_d = nc.declare_dram_parameter(
        "gt", [128, nw * (WIN // 16)], mybir.dt.int16, isOutput=False
    )
    st_d = nc.declare_dram_parameter(
        "st", [128, nw * (WIN // 16)], mybir.dt.int16, isOutput=False
    )
    mk_d = nc.declare_dram_parameter(
        "mk", [128, nw * WIN], mybir.dt.float8e4, isOutput=False
    )
    sums_d = [
        nc.declare_dram_parameter(
            f"sums{k}", [S_EXT, ROW], mybir.dt.float32, isOutput=True
        )
        for k in range(K)
    ]
    JT = WIN // 128
    with tile.TileContext(nc) as tc:
        with (
            tc.tile_pool(name="const", bufs=1) as cpool,
            tc.tile_pool(name="work", bufs=4) as wpool,
        ):
            for w in range(nw):
                gi = wpool.tile([128, WIN // 16], mybir.dt.int16, tag="gi")
                si = wpool.tile([128, WIN // 16], mybir.dt.int16, tag="si")
                nc.sync.dma_start(
                    gi[:], gt_d[:, w * (WIN // 16) : (w + 1) * (WIN // 16)]
                )
                nc.sync.dma_start(
                    si[:], st_d[:, w * (WIN // 16) : (w + 1) * (WIN // 16)]
                )
                g = wpool.tile([128, JT, E], mybir.dt.float32, tag="g")
                nc.gpsimd.dma_gather(g[:], emb_d[:], gi[:], WIN, WIN, E, single_packet=False)
                m8 = wpool.tile([128, WIN], mybir.dt.float8e4, tag="m8")
                nc.sync.dma_start(m8[:], mk_d[:, w * WIN : (w + 1) * WIN])
                mf = wpool.tile([128, WIN], mybir.dt.float32, tag="mf")
                nc.vector.tensor_copy(out=mf[:], in_=m8[:])
                z = wpool.tile([128, JT, ROW], mybir.dt.float32, tag="z")
                nc.vector.memset(z[:, :, E : E + 1], 1.0)
                nc.vector.memset(z[:, :, E + 1 :], 0.0)
                nc.vector.tensor_tensor(
                    out=z[:, :, 0:E],
                    in0=g[:],
                    in1=mf[:].rearrange("p (j e) -> p j e", j=JT),
                    op=mybir.AluOpType.mult,
                )
                nc.gpsimd.dma_scatter_add(
                    sums_d[w % K][:], z[:], si[:], WIN, WIN, ROW,
                    single_packet=False,
                )
    nc.finalize()
    return nc


def _build_launch2(nparts: int):
    import concourse.mybir as mybir
    from concourse import tile
    from concourse.masks import make_identity

    nc = make_nc()
    parts_d = nc.declare_dram_parameter(
        "parts", [nparts, SEG_C, E], mybir.dt.float32, isOutput=False
    )
    cpart_d = nc.declare_dram_parameter(
        "cparts", [SEG_C, nparts], mybir.dt.float32, isOutput=False
    )
    wt_d = nc.declare_dram_parameter("wt", [E, O], mybir.dt.float32, isOutput=False)
    b_d = nc.declare_dram_parameter("bias", [O, 1], mybir.dt.float32, isOutput=False)
    out_d = nc.declare_dram_parameter("outT", [O, SEG_C], mybir.dt.float32, isOutput=True)

    ntile = SEG_C // 128
    with tile.TileContext(nc) as tc:
        with (
            tc.tile_pool(name="const", bufs=1) as cpool,
            tc.tile_pool(name="work", bufs=3) as wpool,
            tc.tile_pool(name="psum", bufs=3, space="PSUM") as ppool,
        ):
            ident = cpool.tile([128, 128], mybir.dt.float32)
            make_identity(nc, ident[:])
            wt_sb = cpool.tile([E, O], mybir.dt.float32)
            nc.sync.dma_start(wt_sb[:], wt_d[:])
            b_sb = cpool.tile([O, 1], mybir.dt.float32)
            nc.sync.dma_start(b_sb[:], b_d[:])
            for i in range(ntile):
                sl = slice(i * 128, (i + 1) * 128)
                acc = wpool.tile([128, E], mybir.dt.float32, tag="acc")
                nc.sync.dma_start(acc[:], parts_d[0, sl, :])
                for k in range(1, nparts):
                    pt = wpool.tile([128, E], mybir.dt.float32, tag="pt")
                    nc.sync.dma_start(pt[:], parts_d[k, sl, :])
                    nc.vector.tensor_add(out=acc[:], in0=acc[:], in1=pt[:])
                ct = wpool.tile([128, nparts], mybir.dt.float32, tag="ct")
                nc.sync.dma_start(ct[:], cpart_d[sl, :])
                csum = wpool.tile([128, 1], mybir.dt.float32, tag="csum")
                nc.vector.tensor_reduce(
                    out=csum[:],
                    in_=ct[:],
                    axis=mybir.AxisListType.X,
                    op=mybir.AluOpType.add,
                )
                nc.vector.tensor_scalar_max(csum[:], csum[:], 1.0)
                rec = wpool.tile([128, 1], mybir.dt.float32, tag="rec")
                nc.vector.reciprocal(rec[:], csum[:])
                nc.vector.tensor_scalar_mul(acc[:], acc[:], rec[:])
                mt_ps = ppool.tile([128, 128], mybir.dt.float32, tag="mt")
                nc.tensor.transpose(out=mt_ps[:], in_=acc[:], identity=ident[:])
                mt = wpool.tile([128, 128], mybir.dt.float32, tag="mts")
                nc.vector.tensor_copy(out=mt[:], in_=mt_ps[:])
                ot_ps = ppool.tile([128, 128], mybir.dt.float32, tag="ot")
                nc.tensor.matmul(
                    out=ot_ps[:], lhsT=wt_sb[:], rhs=mt[:], start=True, stop=True
                )
                ot = wpool.tile([128, 128], mybir.dt.float32, tag="ots")
                nc.vector.tensor_scalar_add(ot[:], ot_ps[:], b_sb[:])
                nc.sync.dma_start(out_d[:, sl], ot[:])
    nc.finalize()
    return nc


def kernel(features, indices, num_segments, emb_table, W, b):
    from concourse.bass_utils import run_bass_kernel_spmd

    features = np.asarray(features)
    indices = np.asarray(indices)
    emb_table = np.asarray(emb_table, dtype=np.float32)
    W = np.asarray(W, dtype=np.float32)
    b = np.asarray(b, dtype=np.float32)

    mask = _mask_fp8()
    nw, routed = _route_tokens(features, indices)
    nw = max(nw, 1)

    if ("l1", nw) not in _prog_cache:
        _prog_cache[("l1", nw)] = _build_launch1(nw)
    nc1 = _prog_cache[("l1", nw)]

    in_maps = []
    for c in range(NCORES):
        toks, gi, si = routed[c]
        mk = np.zeros((nw * WIN, E), _ml_fp8())
        real = toks >= 0
        mk[real] = mask[toks[real]]
        # out-slot layout: window w, slot (p, j): token flat i = w*1024 + j*128 + p
        mk_dev = (
            mk.reshape(nw, WIN // 128, 128, E)
            .transpose(2, 0, 1, 3)
            .reshape(128, nw * WIN)
        )
        in_maps.append(
            {
                "embs": emb_table[c * VS : (c + 1) * VS],
                "gt": _to_table(gi),
                "st": _to_table(si),
                "mk": np.ascontiguousarray(mk_dev),
            }
        )
    res1 = run_bass_kernel_spmd(nc1, in_maps, list(range(NCORES)))

    nparts = NCORES * K
    if ("l2", nparts) not in _prog_cache:
        _prog_cache[("l2", nparts)] = _build_launch2(nparts)
    nc2 = _prog_cache[("l2", nparts)]

    in_maps2 = []
    for c in range(NCORES):
        sl = slice(c * SEG_C, (c + 1) * SEG_C)
        parts = np.stack(
            [
                res1.results[s][f"sums{k}"][sl, :E]
                for s in range(NCORES)
                for k in range(K)
            ]
        )
        cparts = np.stack(
            [
                res1.results[s][f"sums{k}"][sl, E]
                for s in range(NCORES)
                for k in range(K)
            ],
            axis=1,
        )
        in_maps2.append(
            {
                "parts": np.ascontiguousarray(parts),
                "cparts": np.ascontiguousarray(cparts),
                "wt": np.ascontiguousarray(W.T),
                "bias": np.ascontiguousarray(b[:, None]),
            }
        )
    res2 = run_bass_kernel_spmd(nc2, in_maps2, list(range(NCORES)))

    out = np.concatenate(
        [res2.results[c]["outT"].T for c in range(NCORES)], axis=0
    )
    return out.astype(np.float32)


# revision 10
# speedup vs baseline: 1.5724x; 1.5724x over previous
"""BagOfFeaturesModel Trainium2 kernel (8 NeuronCores, SPMD).

Strategy (tensor-parallel vocab-range sharding, per the spec hint's second
variant): core c owns vocab rows [c*12500, (c+1)*12500).  The host routes each
token to the core owning its feature (pure data movement), and within each
core orders tokens by (occurrence-rank within segment, segment), padding each
rank group to a whole 1024-token window.  That makes every 1024-token scatter
window duplicate-free, so the Q7 dma_scatter_add ucode (whose CCE
read-modify-write races on duplicate indices *within* a call) is exact; K
rotating accumulator copies keep cross-window adds race-free while still
overlapping.

Launch 1 (per core): dma_gather (int16 idx) from the 12500-row vocab shard ->
fp8 dropout-mask multiply (mask = jax threefry key(42), input-independent) ->
one dma_scatter_add per window of 192-float rows [z | count=1.0 | pad]
into K rotating sums copies (counts ride along in column 128).

Launch 2 (per core): reduce the 8*K partial copies for the core's 2048-segment
range, mean = sums/max(counts,1), out^T = W @ mean^T + b on the PE, returning
[128, 2048]; the host reassembles [16384, 128].
"""
import functools
import sys

import numpy as np

if "/opt/trn_rl_repo" not in sys.path:
    sys.path.insert(0, "/opt/trn_rl_repo")


def make_nc(**kw):
    import concourse.bacc as bacc

    return bacc.Bacc(**kw)


T = 2_000_000
V = 100_000
E = 128
S = 16_384
O = 128
NCORES = 8
VS = V // NCORES  # 12_500 vocab rows per core
WIN = 1024  # tokens per gather/scatter window (ucode-safe call size)
K = 4  # rotating accumulator copies
SEG_C = S // NCORES  # 2048 segments per core in launch 2
S_EXT = S + WIN  # scatter space incl. per-window dump rows for padding
ROW = 192  # scatter row: 128 emb + 1 count + 63 pad (768 B, /256 aligned)

_prog_cache = {}


def _ml_fp8():
    import ml_dtypes

    return ml_dtypes.float8_e4m3


@functools.lru_cache(maxsize=1)
def _mask_fp8():
    """Reference dropout mask (threefry key 42, keep-prob 0.5) scaled by 2,
    as float8_e4m3 {0.0, 2.0}. Input-independent constant of the problem."""
    import jax

    with jax.default_device(jax.devices("cpu")[0]):
        keep = jax.random.bernoulli(jax.random.key(42), 0.5, (T, E))
        keep = np.asarray(keep)
    return (keep.astype(np.float32) * 2.0).astype(_ml_fp8())


def _to_table(flat16: np.ndarray) -> np.ndarray:
    """Token flat-i -> idx table slot [i % 16, i // 16], replicated 8x to 128
    partitions (one copy per Q7 core)."""
    n = flat16.shape[0]
    t16 = flat16.reshape(n // 16, 16).T.copy()
    return np.tile(t16, (8, 1))


def _route_tokens(features: np.ndarray, indices: np.ndarray):
    """Assign tokens to cores by vocab range; order each core's tokens by
    (rank-within-segment, segment); pad rank groups to WIN multiples.

    Returns (nw, per-core list of (tok_positions, gidx16, sidx16)) where
    padded slots have tok_position == -1, gidx 0, sidx -1."""
    f = features.astype(np.int64)
    g = indices.astype(np.int64)
    core = f // VS
    order0 = np.lexsort((g, core))  # stable: by core, then segment
    cs = core[order0]
    ss = g[order0]
    newrun = np.r_[True, (cs[1:] != cs[:-1]) | (ss[1:] != ss[:-1])]
    idx = np.arange(T)
    starts = np.maximum.accumulate(np.where(newrun, idx, 0))
    rank = idx - starts
    # final order: core, then rank, then segment (stable keeps seg order)
    fin = np.lexsort((rank, cs))
    order = order0[fin]
    cs2 = cs[fin]
    rk2 = rank[fin]

    per_core = []
    lens = []
    for c in range(NCORES):
        sel = cs2 == c
        toks = order[sel]
        rks = rk2[sel]
        # split into rank groups, pad each to WIN multiple
        pieces = []
        if toks.size:
            bounds = np.r_[0, np.nonzero(rks[1:] != rks[:-1])[0] + 1, toks.size]
            for a, b in zip(bounds[:-1], bounds[1:]):
                grp = toks[a:b]
                pad = (-grp.size) % WIN
                if pad:
                    grp = np.r_[grp, np.full(pad, -1, np.int64)]
                pieces.append(grp)
        toks_p = np.concatenate(pieces) if pieces else np.zeros(0, np.int64)
        per_core.append(toks_p)
        lens.append(toks_p.size)
    nw = (max(lens) + WIN - 1) // WIN
    out = []
    for c in range(NCORES):
        toks_p = per_core[c]
        full = np.full(nw * WIN, -1, np.int64)
        full[: toks_p.size] = toks_p
        real = full >= 0
        gi = np.zeros(nw * WIN, np.int16)
        gi[real] = (f[full[real]] - c * VS).astype(np.int16)
        si = (S + (np.arange(nw * WIN) % WIN)).astype(np.int16)
        si[real] = g[full[real]].astype(np.int16)
        out.append((full, gi, si))
    return nw, out


def _build_launch1(nw: int):
    import concourse.mybir as mybir
    from concourse import tile

    nc = make_nc(num_swdge_queues=2)
    emb_d = nc.declare_dram_parameter("embs", [VS, E], mybir.dt.float32, isOutput=False)
    gt_d = nc.declare_dram_parameter(
        "gt", [128, nw * (WIN // 16)], mybir.dt.int16, isOutput=False
    )
    st_d = nc.declare_dram_parameter(
        "st", [128, nw * (WIN // 16)], mybir.dt.int16, isOutput=False
    )
    mk_d = nc.declare_dram_parameter(
        "mk", [128, nw * WIN], mybir.dt.float8e4, isOutput=False
    )
    sums_d = [
        nc.declare_dram_parameter(
            f"sums{k}", [S_EXT, ROW], mybir.dt.float32, isOutput=True
        )
        for k in range(K)
    ]
    JT = WIN // 128
    with tile.TileContext(nc) as tc:
        with (
            tc.tile_pool(name="const", bufs=1) as cpool,
            tc.tile_pool(name="work", bufs=4) as wpool,
        ):
            for w in range(nw):
                gi = wpool.tile([128, WIN // 16], mybir.dt.int16, tag="gi")
                si = wpool.tile([128, WIN // 16], mybir.dt.int16, tag="si")
                nc.sync.dma_start(
                    gi[:], gt_d[:, w * (WIN // 16) : (w + 1) * (WIN // 16)]
                )
                nc.sync.dma_start(
                    si[:], st_d[:, w * (WIN // 16) : (w + 1) * (WIN // 16)]
                )
                g = wpool.tile([128, JT, E], mybir.dt.float32, tag="g")
                nc.gpsimd.dma_gather(g[:], emb_d[:], gi[:], WIN, WIN, E, queue_num=w % 2)
                m8 = wpool.tile([128, WIN], mybir.dt.float8e4, tag="m8")
                nc.sync.dma_start(m8[:], mk_d[:, w * WIN : (w + 1) * WIN])
                mf = wpool.tile([128, WIN], mybir.dt.float32, tag="mf")
                nc.vector.tensor_copy(out=mf[:], in_=m8[:])
                z = wpool.tile([128, JT, ROW], mybir.dt.float32, tag="z")
                nc.vector.memset(z[:, :, E : E + 1], 1.0)
                nc.vector.memset(z[:, :, E + 1 :], 0.0)
                nc.vector.tensor_tensor(
                    out=z[:, :, 0:E],
                    in0=g[:],
                    in1=mf[:].rearrange("p (j e) -> p j e", j=JT),
                    op=mybir.AluOpType.mult,
                )
                nc.gpsimd.dma_scatter_add(
                    sums_d[w % K][:], z[:], si[:], WIN, WIN, ROW, queue_num=w % 2
                )
    nc.finalize()
    return nc


def _build_launch2(nparts: int):
    import concourse.mybir as mybir
    from concourse import tile
    from concourse.masks import make_identity

    nc = make_nc()
    parts_d = nc.declare_dram_parameter(
        "parts", [nparts, SEG_C, E], mybir.dt.float32, isOutput=False
    )
    cpart_d = nc.declare_dram_parameter(
        "cparts", [SEG_C, nparts], mybir.dt.float32, isOutput=False
    )
    wt_d = nc.declare_dram_parameter("wt", [E, O], mybir.dt.float32, isOutput=False)
    b_d = nc.declare_dram_parameter("bias", [O, 1], mybir.dt.float32, isOutput=False)
    out_d = nc.declare_dram_parameter("outT", [O, SEG_C], mybir.dt.float32, isOutput=True)

    ntile = SEG_C // 128
    with tile.TileContext(nc) as tc:
        with (
            tc.tile_pool(name="const", bufs=1) as cpool,
            tc.tile_pool(name="work", bufs=3) as wpool,
            tc.tile_pool(name="psum", bufs=3, space="PSUM") as ppool,
        ):
            ident = cpool.tile([128, 128], mybir.dt.float32)
            make_identity(nc, ident[:])
            wt_sb = cpool.tile([E, O], mybir.dt.float32)
            nc.sync.dma_start(wt_sb[:], wt_d[:])
            b_sb = cpool.tile([O, 1], mybir.dt.float32)
            nc.sync.dma_start(b_sb[:], b_d[:])
            for i in range(ntile):
                sl = slice(i * 128, (i + 1) * 128)
                acc = wpool.tile([128, E], mybir.dt.float32, tag="acc")
                nc.sync.dma_start(acc[:], parts_d[0, sl, :])
                for k in range(1, nparts):
                    pt = wpool.tile([128, E], mybir.dt.float32, tag="pt")
                    nc.sync.dma_start(pt[:], parts_d[k, sl, :])
                    nc.vector.tensor_add(out=acc[:], in0=acc[:], in1=pt[:])
                ct = wpool.tile([128, nparts], mybir.dt.float32, tag="ct")
                nc.sync.dma_start(ct[:], cpart_d[sl, :])
                csum = wpool.tile([128, 1], mybir.dt.float32, tag="csum")
                nc.vector.tensor_reduce(
                    out=csum[:],
                    in_=ct[:],
                    axis=mybir.AxisListType.X,
                    op=mybir.AluOpType.add,
                )
                nc.vector.tensor_scalar_max(csum[:], csum[:], 1.0)
                rec = wpool.tile([128, 1], mybir.dt.float32, tag="rec")
                nc.vector.reciprocal(rec[:], csum[:])
                nc.vector.tensor_scalar_mul(acc[:], acc[:], rec[:])
                mt_ps = ppool.tile([128, 128], mybir.dt.float32, tag="mt")
                nc.tensor.transpose(out=mt_ps[:], in_=acc[:], identity=ident[:])
                mt = wpool.tile([128, 128], mybir.dt.float32, tag="mts")
                nc.vector.tensor_copy(out=mt[:], in_=mt_ps[:])
                ot_ps = ppool.tile([128, 128], mybir.dt.float32, tag="ot")
                nc.tensor.matmul(
                    out=ot_ps[:], lhsT=wt_sb[:], rhs=mt[:], start=True, stop=True
                )
                ot = wpool.tile([128, 128], mybir.dt.float32, tag="ots")
                nc.vector.tensor_scalar_add(ot[:], ot_ps[:], b_sb[:])
                nc.sync.dma_start(out_d[:, sl], ot[:])
    nc.finalize()
    return nc


def kernel(features, indices, num_segments, emb_table, W, b):
    from concourse.bass_utils import run_bass_kernel_spmd

    features = np.asarray(features)
    indices = np.asarray(indices)
    emb_table = np.asarray(emb_table, dtype=np.float32)
    W = np.asarray(W, dtype=np.float32)
    b = np.asarray(b, dtype=np.float32)

    mask = _mask_fp8()
    nw, routed = _route_tokens(features, indices)
    nw = max(nw, 1)

    if ("l1", nw) not in _prog_cache:
        _prog_cache[("l1", nw)] = _build_launch1(nw)
    nc1 = _prog_cache[("l1", nw)]

    in_maps = []
    for c in range(NCORES):
        toks, gi, si = routed[c]
        mk = np.zeros((nw * WIN, E), _ml_fp8())
        real = toks >= 0
        mk[real] = mask[toks[real]]
        # out-slot layout: window w, slot (p, j): token flat i = w*1024 + j*128 + p
        mk_dev = (
            mk.reshape(nw, WIN // 128, 128, E)
            .transpose(2, 0, 1, 3)
            .reshape(128, nw * WIN)
        )
        in_maps.append(
            {
                "embs": emb_table[c * VS : (c + 1) * VS],
                "gt": _to_table(gi),
                "st": _to_table(si),
                "mk": np.ascontiguousarray(mk_dev),
            }
        )
    res1 = run_bass_kernel_spmd(nc1, in_maps, list(range(NCORES)))

    nparts = NCORES * K
    if ("l2", nparts) not in _prog_cache:
        _prog_cache[("l2", nparts)] = _build_launch2(nparts)
    nc2 = _prog_cache[("l2", nparts)]

    in_maps2 = []
    for c in range(NCORES):
        sl = slice(c * SEG_C, (c + 1) * SEG_C)
        parts = np.stack(
            [
                res1.results[s][f"sums{k}"][sl, :E]
                for s in range(NCORES)
                for k in range(K)
            ]
        )
        cparts = np.stack(
            [
                res1.results[s][f"sums{k}"][sl, E]
                for s in range(NCORES)
                for k in range(K)
            ],
            axis=1,
        )
        in_maps2.append(
            {
                "parts": np.ascontiguousarray(parts),
                "cparts": np.ascontiguousarray(cparts),
                "wt": np.ascontiguousarray(W.T),
                "bias": np.ascontiguousarray(b[:, None]),
            }
        )
    res2 = run_bass_kernel_spmd(nc2, in_maps2, list(range(NCORES)))

    out = np.concatenate(
        [res2.results[c]["outT"].T for c in range(NCORES)], axis=0
    )
    return out.astype(np.float32)
